# revision 12
# baseline (speedup 1.0000x reference)
"""Causal GQA attention (B=2, L=2048, D=2048, H=16, KV=4, K=128) on 8 trn2 cores.

Sharding: core = b*4 + g  (b: batch 0..1, g: GQA group 0..3).
Each core computes, for its batch b and its 4 Q heads / 1 KV head:
    q/k/v projections -> rope -> causal attention -> partial out-projection
and writes yT_partial = (partial y).T to DRAM (bf16). Host sums the 4 group
partials per batch and transposes back.

v3 vs v2:
 - flat single-DMA weight layouts ([128, ND*oc] contiguous per partition);
   bf16 cos/sin; x chunks spread over 4 trigger queues in priority order
   (weights first, wo last)
 - phase A as per-oc blocks over 8 PSUM tags (k and v stream x on arrival,
   q blocks pipeline behind; rope drain of block i overlaps block i+1)
 - causal diagonal: columns left of the 128-wide diagonal band are fully
   masked -> skipped in scores/attnV matmuls, exp, and acc; mask is one
   fixed [128,128] band added only to the band region
 - rowsum: ones-matmul with 128 ones-columns broadcasts the row sums to all
   partitions in PSUM; reciprocal reads it directly (no [1,512] copy, no
   gpsimd partition_broadcast)
"""

import sys

if "/opt/trn_rl_repo" not in sys.path:
    sys.path.insert(0, "/opt/trn_rl_repo")

import numpy as np

B, L, D, H, KV = 2, 2048, 2048, 16, 4
K = D // H          # 128 head dim
G = H // KV         # 4 q heads per kv head
NH = G              # q heads per core
LT = 512            # seq tile (moving operand width)
NLT = L // LT       # 4
ND = D // 128       # 16 contraction chunks
NJ = D // 128       # 16 output-column chunks
ROPE_BASE = 10000.0
MASK_VAL = -30000.0

_NC_CACHE = {}


def _build_nc():
    import concourse.bacc as bacc
    import concourse.mybir as mybir
    from concourse.tile import TileContext

    f32 = mybir.dt.float32
    bf16 = mybir.dt.bfloat16
    EXP = mybir.ActivationFunctionType.Exp
    nc = bacc.Bacc("TRN2", target_bir_lowering=False, debug=False, num_devices=8)

    # ---- DRAM parameters (host-pre-tiled layouts, bf16) ----
    xT = nc.dram_tensor("xT", [ND, 128, L], bf16, kind="ExternalInput")
    wqF = nc.dram_tensor("wqF", [128, ND * 512], bf16, kind="ExternalInput")
    wkF = nc.dram_tensor("wkF", [128, ND * 128], bf16, kind="ExternalInput")
    wvF = nc.dram_tensor("wvF", [128, ND * 128], bf16, kind="ExternalInput")
    woT = nc.dram_tensor("woT", [128, NH * NJ * 128], bf16, kind="ExternalInput")
    cosT = nc.dram_tensor("cosT", [128, L], bf16, kind="ExternalInput")
    sinT = nc.dram_tensor("sinT", [128, L], bf16, kind="ExternalInput")
    bandT = nc.dram_tensor("bandT", [128, 128], f32, kind="ExternalInput")
    pswap = nc.dram_tensor("pswap", [128, 128], bf16, kind="ExternalInput")
    onesc = nc.dram_tensor("onesc", [128, 128], bf16, kind="ExternalInput")
    ident = nc.dram_tensor("ident", [128, 128], bf16, kind="ExternalInput")
    yT = nc.dram_tensor("yT", [NJ, NLT, 128, LT], bf16, kind="ExternalOutput")

    with TileContext(nc) as tc:
        p_const = tc.alloc_tile_pool(name="const", bufs=1)
        p_w = tc.alloc_tile_pool(name="w", bufs=1)
        p_x = tc.alloc_tile_pool(name="xres", bufs=1)
        p_vraw = tc.alloc_tile_pool(name="vraw", bufs=1)
        p_rope = tc.alloc_tile_pool(name="ropeout", bufs=1)
        p_qs = tc.alloc_tile_pool(name="qs", bufs=4)
        p_tmp = tc.alloc_tile_pool(name="tmp", bufs=4)
        p_pt = tc.alloc_tile_pool(name="pt", bufs=4)
        p_acc = tc.alloc_tile_pool(name="acc", bufs=3)
        p_on = tc.alloc_tile_pool(name="on", bufs=2)
        p_bc = tc.alloc_tile_pool(name="bc", bufs=2)
        p_ysb = tc.alloc_tile_pool(name="ysb", bufs=3)

        # ---- DMA (emission order = scheduler priority) ----
        # sync + scalar are HWDGE (fast issue); gpsimd/vector are SWDGE.
        pswap_sb = p_const.tile([128, 128], bf16, tag="pswap", name="pswap")
        nc.sync.dma_start(out=pswap_sb[:], in_=pswap.ap())
        ones_sb = p_const.tile([128, 128], bf16, tag="ones", name="ones")
        nc.sync.dma_start(out=ones_sb[:], in_=onesc.ap())
        band_sb = p_const.tile([128, 128], f32, tag="band", name="band")
        nc.sync.dma_start(out=band_sb[:], in_=bandT.ap())
        ident_sb = p_const.tile([128, 128], bf16, tag="ident", name="ident")
        nc.sync.dma_start(out=ident_sb[:], in_=ident.ap())

        # one dma_start is serviced by one DMA engine (~27 GB/s) — split every
        # tensor into ~128-256KB slices so transfers parallelize across engines
        wk_sb = p_w.tile([128, ND * 128], bf16, tag="wk", name="wk")
        for s in range(4):
            w = ND * 128 // 4
            nc.sync.dma_start(out=wk_sb[:, s * w:(s + 1) * w],
                              in_=wkF.ap()[:, s * w:(s + 1) * w])
        cos_sb = p_const.tile([128, L], bf16, tag="cos", name="cos")
        sin_sb = p_const.tile([128, L], bf16, tag="sin", name="sin")
        for s in range(2):
            w = L // 2
            nc.sync.dma_start(out=cos_sb[:, s * w:(s + 1) * w],
                              in_=cosT.ap()[:, s * w:(s + 1) * w])
            nc.sync.dma_start(out=sin_sb[:, s * w:(s + 1) * w],
                              in_=sinT.ap()[:, s * w:(s + 1) * w])

        wq_sb = p_w.tile([128, ND * 512], bf16, tag="wq", name="wq")
        for s in range(ND):
            nc.scalar.dma_start(out=wq_sb[:, s * 512:(s + 1) * 512],
                                in_=wqF.ap()[:, s * 512:(s + 1) * 512])
        wv_sb = p_w.tile([128, ND * 128], bf16, tag="wv", name="wv")
        for s in range(4):
            w = ND * 128 // 4
            nc.scalar.dma_start(out=wv_sb[:, s * w:(s + 1) * w],
                                in_=wvF.ap()[:, s * w:(s + 1) * w])

        # resident x: [128, d*2048 + l*512 + i]; 32 (d, l-half) slices,
        # L-HALF-MAJOR order so all-d of l-tiles 0,1 land first and every
        # projection block can start before x is fully resident
        x_sb = p_x.tile([128, ND * L], bf16, tag="x", name="x")
        hw_ = L // 2
        for hf in range(2):
            xq = [nc.gpsimd, nc.sync] if hf == 0 else [nc.gpsimd, nc.scalar]
            for d in range(ND):
                off = d * L + hf * hw_
                xq[d % 2].dma_start(out=x_sb[:, off: off + hw_],
                                    in_=xT.ap()[d][:, hf * hw_:(hf + 1) * hw_])

        wo_sb = p_w.tile([128, NH * NJ * 128], bf16, tag="wo", name="wo")
        for s in range(8):
            w = NH * NJ * 128 // 8
            nc.gpsimd.dma_start(out=wo_sb[:, s * w:(s + 1) * w],
                                in_=woT.ap()[:, s * w:(s + 1) * w])

        # rope outputs + v
        vrawT = p_vraw.tile([128, L], bf16, tag="vrawT", name="vrawT")
        qrope = [p_rope.tile([128, L], bf16, tag=f"qrope{h}", name=f"qrope{h}")
                 for h in range(NH)]
        krope = p_rope.tile([128, L], bf16, tag="krope", name="krope")
        v_sb = p_rope.tile([128, L], bf16, tag="v", name="v")

        # ---- phase A: single-bank (oc, l) blocks, l-major, 8 rotating tags.
        # Up to 8 blocks in flight: all 6 projections advance on l-tile t
        # while x for tile t+1 is still arriving. v transposes ride the same
        # rotation right after each v-block drains.
        psA = tc.alloc_tile_pool(name="psA", bufs=1, space="PSUM")
        ocs = ["k", "v", "q0", "q1", "q2", "q3"]
        blk = 0

        def next_tag():
            nonlocal blk
            t = blk % 8
            blk += 1
            return f"pa{t}"

        for l in range(NLT):
            lsl = slice(l * LT, (l + 1) * LT)
            for i, oc in enumerate(ocs):
                ps = psA.tile([128, LT], f32, tag=next_tag(), name=f"pj{oc}{l}")
                for d in range(ND):
                    if oc == "k":
                        w_ap = wk_sb[:, d * 128:(d + 1) * 128]
                    elif oc == "v":
                        w_ap = wv_sb[:, d * 128:(d + 1) * 128]
                    else:
                        qh = i - 2
                        w_ap = wq_sb[:, d * 512 + qh * 128: d * 512 + (qh + 1) * 128]
                    nc.tensor.matmul(
                        ps[:], w_ap,
                        x_sb[:, d * L + l * LT: d * L + (l + 1) * LT],
                        start=(d == 0), stop=(d == ND - 1))
                if oc == "v":
                    nc.scalar.copy(vrawT[:, lsl], ps[:])
                    for cc in range(4):
                        c = l * 4 + cc
                        pvt = psA.tile([128, 128], bf16, tag=next_tag(),
                                       name=f"vt{c}")
                        nc.tensor.transpose(pvt[:], vrawT[:, c * 128:(c + 1) * 128],
                                            ident_sb[:])
                        if cc % 2 == 0:
                            nc.scalar.copy(v_sb[:, c * 128:(c + 1) * 128], pvt[:])
                        else:
                            nc.vector.tensor_copy(v_sb[:, c * 128:(c + 1) * 128],
                                                  pvt[:])
                    continue
                dst = krope if oc == "k" else qrope[i - 2]
                qs = p_qs.tile([128, LT], bf16, tag="qs", name="qs")
                nc.scalar.copy(qs[:], ps[:])
                psw = psA.tile([128, LT], f32, tag=next_tag(), name=f"psw{oc}{l}")
                nc.tensor.matmul(psw[:], pswap_sb[:], qs[:], start=True, stop=True)
                t1 = p_tmp.tile([128, LT], f32, tag="t1", name="t1")
                nc.vector.tensor_mul(t1[:], qs[:], cos_sb[:, lsl])
                t2 = p_tmp.tile([128, LT], f32, tag="t2", name="t2")
                nc.vector.tensor_mul(t2[:], psw[:], sin_sb[:, lsl])
                nc.vector.tensor_add(dst[:, lsl], t1[:], t2[:])
        psA.release()

        # ---- phases C+D fused per lq-tile ----
        psS = tc.alloc_tile_pool(name="psS", bufs=3, space="PSUM")
        psO = tc.alloc_tile_pool(name="psO", bufs=2, space="PSUM")
        psSUM = tc.alloc_tile_pool(name="psSUM", bufs=1, space="PSUM")
        psY = tc.alloc_tile_pool(name="psY", bufs=2, space="PSUM")
        for jq in range(NLT):
            onorm = []
            for h in range(NH):
                nch = 4 * (jq + 1)
                po = psO.tile([128, LT], f32, tag="po", name="po")
                acc = p_acc.tile([128, LT], bf16, tag="acc", name="acc")
                for c in range(nch):
                    r = c - 4 * jq  # diagonal band index; >=0 in last 4 chunks
                    off = 128 * r if r > 0 else 0
                    ps = psS.tile([128, LT], f32, tag="ps", name="ps")
                    nc.tensor.matmul(
                        ps[:, off:LT], krope[:, c * 128:(c + 1) * 128],
                        qrope[h][:, jq * LT + off:(jq + 1) * LT],
                        start=True, stop=True)
                    if r >= 0:
                        nc.vector.tensor_add(ps[:, off:off + 128],
                                             ps[:, off:off + 128], band_sb[:])
                    pt = p_pt.tile([128, LT], bf16, tag="pt", name="pt")
                    nc.scalar.activation(pt[:, off:LT], ps[:, off:LT], EXP)
                    nc.tensor.matmul(po[:, off:LT], v_sb[:, c * 128:(c + 1) * 128],
                                     pt[:, off:LT], start=(c == 0),
                                     stop=(c == nch - 1), skip_group_check=True)
                    if c == 0:
                        nc.vector.tensor_copy(acc[:], pt[:])
                    elif off == 0:
                        nc.vector.tensor_add(acc[:], acc[:], pt[:])
                    else:
                        nc.vector.tensor_add(acc[:, off:LT], acc[:, off:LT],
                                             pt[:, off:LT])
                # broadcast rowsums: every output row = sum_k acc[k, q]
                psm = psSUM.tile([128, LT], f32, tag="pm", name="pm")
                nc.tensor.matmul(psm[:], ones_sb[:], acc[:], start=True, stop=True)
                bc = p_bc.tile([128, LT], f32, tag="bc", name="bc")
                nc.vector.reciprocal_approx_fast(bc[:], psm[:])
                on = p_on.tile([128, LT], bf16, tag=f"on{h}", name=f"on{h}")
                nc.vector.tensor_mul(on[:], po[:], bc[:])
                onorm.append(on)
            # output projection for this lq-tile (resident wo)
            for j in range(NJ):
                py = psY.tile([128, LT], f32, tag="py", name="py")
                for h in range(NH):
                    nc.tensor.matmul(
                        py[:], wo_sb[:, (h * NJ + j) * 128:(h * NJ + j + 1) * 128],
                        onorm[h][:], start=(h == 0), stop=(h == NH - 1))
                yt = p_ysb.tile([128, LT], bf16, tag="yt", name="yt")
                if j % 2 == 0:
                    nc.vector.tensor_copy(yt[:], py[:])
                else:
                    nc.scalar.copy(yt[:], py[:])
                nc.sync.dma_start(out=yT.ap()[j, jq], in_=yt[:])
        psY.release()
        psSUM.release()
        psO.release()
        psS.release()
        for pool in (p_ysb, p_bc, p_on, p_acc, p_pt, p_tmp, p_qs,
                     p_rope, p_vraw, p_x, p_w, p_const):
            pool.release()

    nc.compile()
    return nc


def _get_nc():
    if "nc" not in _NC_CACHE:
        import concourse.mybir as mybir  # noqa: F401
        _NC_CACHE["nc"] = _build_nc()
    return _NC_CACHE["nc"]


def _host_tables():
    import ml_dtypes
    bf = ml_dtypes.bfloat16
    iv = (1.0 / (ROPE_BASE ** (np.arange(0, K, 2, dtype=np.float32) / np.float32(K)))).astype(np.float32)
    t = np.arange(L, dtype=np.float32)
    freqs = np.outer(t, iv).astype(np.float32)          # [L, 64]
    cos = np.cos(freqs).astype(np.float32)
    sin = np.sin(freqs).astype(np.float32)
    cosT = np.empty((128, L), np.float32)
    sinT = np.empty((128, L), np.float32)
    cosT[0::2] = cos.T
    cosT[1::2] = cos.T
    sinT[0::2] = -sin.T
    sinT[1::2] = sin.T

    p = np.arange(128)[:, None]
    f = np.arange(128)[None, :]
    band = np.where(f < p, np.float32(MASK_VAL), np.float32(0.0)).astype(np.float32)

    pswap = np.zeros((128, 128), np.float32)
    idx = np.arange(128)
    pswap[idx ^ 1, idx] = 1.0
    onesc = np.ones((128, 128), np.float32)
    ident = np.eye(128, dtype=np.float32)
    return (cosT.astype(bf), sinT.astype(bf), band, pswap.astype(bf),
            onesc.astype(bf), ident.astype(bf))


def _tile_xT(xb, bf):
    # x[b] [L, D] -> xT tiles [ND, 128, L]: xT[d, l] = x[l, d]
    xT = xb.T.astype(bf)  # [D, L]
    return np.ascontiguousarray(xT.reshape(ND, 128, L))


def _prep_inputs(x, wq, wk, wv, wo):
    import ml_dtypes
    bf = ml_dtypes.bfloat16
    cosT, sinT, band, pswap, onesc, ident = _host_tables()
    scale = np.float32(K) ** np.float32(-0.5)
    in_maps = []
    xts = [_tile_xT(np.ascontiguousarray(x[b]), bf) for b in range(B)]
    for b in range(B):
        for g in range(KV):
            wq_g = (wq[g * 512:(g + 1) * 512, :] * scale).astype(bf)
            # flat [128, d*512 + oc] = wq_g[oc, 128d + p]
            wqF_t = np.ascontiguousarray(
                wq_g.T.reshape(ND, 128, 512).transpose(1, 0, 2)
                .reshape(128, ND * 512))
            wk_g = wk[g * 128:(g + 1) * 128, :].astype(bf)
            wkF_t = np.ascontiguousarray(
                wk_g.T.reshape(ND, 128, 128).transpose(1, 0, 2)
                .reshape(128, ND * 128))
            wv_g = wv[g * 128:(g + 1) * 128, :].astype(bf)
            wvF_t = np.ascontiguousarray(
                wv_g.T.reshape(ND, 128, 128).transpose(1, 0, 2)
                .reshape(128, ND * 128))
            wo_g = wo[:, g * 512:(g + 1) * 512]                    # [D, 512]
            # woT flat [128, (h*NJ+j)*128 + c] = wo[128j+c, 512g+128h+p]
            woT_t = np.ascontiguousarray(
                wo_g.T.reshape(NH, 128, NJ, 128).transpose(1, 0, 2, 3)
                .reshape(128, NH * NJ * 128)).astype(bf)
            in_maps.append({
                "xT": xts[b], "wqF": wqF_t, "wkF": wkF_t, "wvF": wvF_t,
                "woT": woT_t, "cosT": cosT, "sinT": sinT, "bandT": band,
                "pswap": pswap, "onesc": onesc, "ident": ident,
            })
    return in_maps


def _gather(results):
    out = np.empty((B, L, D), np.float32)
    for b in range(B):
        acc = None
        for g in range(KV):
            yt = results[b * KV + g]["yT"].astype(np.float32)  # [NJ, NLT, 128, LT]
            full = yt.transpose(0, 2, 1, 3).reshape(D, L)      # [j, l]
            acc = full if acc is None else acc + full
        out[b] = acc.T
    return out


def run(inputs, trace=False, trace_kwargs=None):
    from concourse.bass_utils import run_bass_kernel_spmd
    nc = _get_nc()
    in_maps = _prep_inputs(**inputs)
    res = run_bass_kernel_spmd(nc, in_maps, list(range(8)), trace=trace,
                               **(trace_kwargs or {}))
    return _gather(res.results), res


def kernel(x, wq, wk, wv, wo):
    out, _ = run({"x": x, "wq": wq, "wk": wk, "wv": wv, "wo": wo})
    return out
